# revision 1
# baseline (speedup 1.0000x reference)
"""Trainium2 Bass kernel for nn_LlamaAttention_61495341744411.

Sharding: tensor-parallel over heads across 8 NeuronCores.
  core c: q heads [4c, 4c+4), kv head c, wo cols [512c, 512c+512).
  Each core computes a full-token partial of out^T; host sums partials.

Device program per core (single SPMD Bass program, bf16 matmuls):
  P1: q/k/v projections from hidden^T (host-transposed), RoPE fused,
      emitted per sequence and interleaved with...
  P2: causal flash prefill attention per (head, seq).
  P3: paged decode attention (32 seqs, kv cache head c).
  P4: o_proj partial: out^T = woT.T @ attn^T.

DMA routing: large streaming transfers on nc.sync (HWDGE, shared exclusive
resource), small/frequent transfers on nc.gpsimd (SWDGE, runs on the
otherwise idle Pool engine) and batched wherever possible.
"""
import sys

if "/opt/trn_rl_repo" not in sys.path:
    sys.path.insert(0, "/opt/trn_rl_repo")

import numpy as np
import ml_dtypes

BF16 = ml_dtypes.bfloat16

PREFILLS = [1024, 1536, 2048, 512]
DOFF = sum(PREFILLS)            # 5120
DECODE = 32
PAST = 2048
HIDDEN = 4096
NQ, NKV, HD = 32, 8, 128
G = NQ // NKV                   # 4
T = DOFF + DECODE               # 5152
SCALE = 1.0 / float(np.sqrt(HD))
NCORES = 8
QH = NQ // NCORES               # 4 q heads per core
ADIM = QH * HD                  # 512
KS = HIDDEN // 128              # 32 contraction subtiles
P = 128
TW = 256                        # projection t-tile width
NKT_D = PAST // P               # 16 decode cache k-tiles

SEQ_BOUNDS = []
_off = 0
for _L in PREFILLS:
    SEQ_BOUNDS.append((_off, _L))
    _off += _L


def build_program():
    import concourse.mybir as mybir
    import concourse.tile as tile
    from concourse import bacc
    from concourse.masks import make_identity
    from contextlib import ExitStack

    dt = mybir.dt
    AF = mybir.ActivationFunctionType
    ALU = mybir.AluOpType
    AX = mybir.AxisListType
    f32 = dt.float32
    bf = dt.bfloat16

    nc = bacc.Bacc(None, target_bir_lowering=False, debug=False)

    hT = nc.dram_tensor("hT", [HIDDEN, T], bf, kind="ExternalInput")
    wqT = nc.dram_tensor("wqT", [HIDDEN, ADIM], bf, kind="ExternalInput")
    wkT = nc.dram_tensor("wkT", [HIDDEN, HD], bf, kind="ExternalInput")
    wvT = nc.dram_tensor("wvT", [HIDDEN, HD], bf, kind="ExternalInput")
    woT = nc.dram_tensor("woT", [ADIM, HIDDEN], bf, kind="ExternalInput")
    kTc = nc.dram_tensor("kTc", [DECODE, HD, PAST], bf, kind="ExternalInput")
    vc = nc.dram_tensor("vc", [DECODE, PAST, HD], bf, kind="ExternalInput")
    qcos = nc.dram_tensor("qcos", [HD, T], bf, kind="ExternalInput")
    qsin = nc.dram_tensor("qsin", [HD, T], bf, kind="ExternalInput")
    dcos = nc.dram_tensor("dcos", [HD, PAST], bf, kind="ExternalInput")
    dsin = nc.dram_tensor("dsin", [HD, PAST], bf, kind="ExternalInput")
    outT = nc.dram_tensor("outT", [HIDDEN, T], bf, kind="ExternalOutput")

    hT_r = hT.rearrange("(o p) t -> p o t", p=P)        # [128, 32, T]
    wqT_r = wqT.rearrange("(o p) m -> p o m", p=P)      # [128, 32, 512]
    wkT_r = wkT.rearrange("(o p) m -> p o m", p=P)      # [128, 32, 128]
    wvT_r = wvT.rearrange("(o p) m -> p o m", p=P)
    woT_r = woT.rearrange("(o p) m -> p o m", p=P)      # [128, 4, 4096]
    outT_r = outT.rearrange("(o p) t -> p o t", p=P)    # [128, 32, T]

    with ExitStack() as ctx:
        tc = ctx.enter_context(tile.TileContext(nc))
        p1 = ctx.enter_context(tc.tile_pool(name="p1", bufs=1))
        p2 = ctx.enter_context(tc.tile_pool(name="p2", bufs=2))
        p3 = ctx.enter_context(tc.tile_pool(name="p3", bufs=3))
        psA = ctx.enter_context(tc.tile_pool(name="psA", bufs=3, space="PSUM"))
        psB = ctx.enter_context(tc.tile_pool(name="psB", bufs=4, space="PSUM"))
        dram = ctx.enter_context(tc.tile_pool(name="dram", bufs=1, space="DRAM"))

        # per-sequence DRAM spill tensors (explicit dep granularity)
        qT_seq = [dram.tile([ADIM, L], bf, name=f"qTs{i}")
                  for i, (_, L) in enumerate(SEQ_BOUNDS)]
        qT_dec = dram.tile([ADIM, DECODE], bf, name="qTdec")
        attnT_seq = [dram.tile([ADIM, L], bf, name=f"aTs{i}")
                     for i, (_, L) in enumerate(SEQ_BOUNDS)]
        attnT_dec = dram.tile([ADIM, DECODE], bf, name="aTdec")

        ident = p1.tile([P, P], bf, tag="ident")
        make_identity(nc, ident)

        # resident weights (wq shares its slot with wo via tag)
        wq_sb = p1.tile([P, KS, ADIM], bf, tag="wbig")
        nc.sync.dma_start(wq_sb[:], wqT_r[:])
        wk_sb = p1.tile([P, KS, HD], bf, tag="wk")
        nc.sync.dma_start(wk_sb[:], wkT_r[:])
        wv_sb = p1.tile([P, KS, HD], bf, tag="wv")
        nc.sync.dma_start(wv_sb[:], wvT_r[:])

        # per-sequence resident activations
        kT_seq = [p1.tile([P, L], bf, tag=f"kT{i}", name=f"kTs{i}")
                  for i, (_, L) in enumerate(SEQ_BOUNDS)]
        v_seq = [p1.tile([P, L // P, HD], bf, tag=f"v{i}", name=f"vs{i}")
                 for i, (_, L) in enumerate(SEQ_BOUNDS)]
        kT_dec = p1.tile([P, DECODE], bf, tag="kTdec")     # roped new decode keys
        vdt = p1.tile([DECODE, HD], bf, tag="vdt")         # decode new v rows
        odec_sb = p1.tile([P, HD], bf, tag="odec")         # decode outs, rows (s,h)
        qdec_sb = p1.tile([P, P], bf, tag="qdec")          # decode q, cols (s,h)

        NB = QH + 1  # q head blocks + k block, roped together

        def phase1_tile(t0, W, kT_dst, kcol0, v_dst, q_dst_r, qcol0):
            """Projections + rope for tokens [t0, t0+W). Writes:
            kT_dst[:, kcol0:kcol0+W], v_dst (v_nat tile or vdt), q_dst_r
            ([128, QH, Lseq] rearranged dram AP) at qcol0."""
            ht = p2.tile([P, KS, TW], bf, tag="ht")
            nc.sync.dma_start(ht[:, :, :W], hT_r[:, :, t0:t0 + W])
            ct = p3.tile([P, TW], bf, tag="cos")
            st = p3.tile([P, TW], bf, tag="sin")
            nc.gpsimd.dma_start(ct[:, :W], qcos[:, t0:t0 + W])
            nc.gpsimd.dma_start(st[:, :W], qsin[:, t0:t0 + W])

            xq = p2.tile([P, NB, TW], bf, tag="xq")
            for m in range(NB):  # 0..3 = q heads, 4 = k
                ps = psA.tile([P, 512], f32, tag="big")
                if m < QH:
                    w_ap = wq_sb[:, :, m * P:(m + 1) * P]
                else:
                    w_ap = wk_sb[:, :, :]
                for ks in range(KS):
                    nc.tensor.matmul(
                        ps[:, :W], lhsT=w_ap[:, ks, :], rhs=ht[:, ks, :W],
                        start=(ks == 0), stop=(ks == KS - 1))
                nc.any.tensor_copy(out=xq[:, m, :W], in_=ps[:, :W])
            rotq = p2.tile([P, NB, TW], bf, tag="rotq")
            nc.gpsimd.dma_start(out=rotq[0:64, :, :W], in_=xq[64:128, :, :W])
            nc.gpsimd.dma_start(out=rotq[64:128, :, :W], in_=xq[0:64, :, :W])
            ct_b = ct[:, None, :W].to_broadcast((P, NB, W))
            st_b = st[:, None, :W].to_broadcast((P, NB, W))
            nc.vector.tensor_tensor(xq[:, :, :W], xq[:, :, :W], ct_b, ALU.mult)
            nc.vector.tensor_tensor(rotq[:, :, :W], rotq[:, :, :W], st_b, ALU.mult)
            nc.vector.tensor_tensor(xq[:, :, :W], xq[:, :, :W], rotq[:, :, :W],
                                    ALU.add)
            nc.sync.dma_start(q_dst_r[:, :, qcol0:qcol0 + W], xq[:, :QH, :W])
            nc.any.tensor_copy(out=kT_dst[:, kcol0:kcol0 + W], in_=xq[:, QH, :W])

            # v projection (no rope)
            ps = psA.tile([P, 512], f32, tag="big")
            for ks in range(KS):
                nc.tensor.matmul(ps[:, :W], lhsT=wv_sb[:, ks, :], rhs=ht[:, ks, :W],
                                 start=(ks == 0), stop=(ks == KS - 1))
            vt = p3.tile([P, TW], bf, tag="vt")
            nc.any.tensor_copy(out=vt[:, :W], in_=ps[:, :W])
            if W == TW:
                for j in range(TW // P):
                    pst = psB.tile([P, P], bf, tag="small")
                    nc.tensor.transpose(pst[:], vt[:, j * P:(j + 1) * P], ident[:])
                    nc.any.tensor_copy(
                        out=v_dst[:, (kcol0 // P) + j, :], in_=pst[:])
            else:  # decode tile, W == 32
                pst = psB.tile([P, P], bf, tag="small")
                nc.tensor.transpose(pst[:DECODE, :], vt[:, :W], ident[:])
                nc.any.tensor_copy(out=v_dst[:], in_=pst[:DECODE, :])

        def phase2_seq(si, h):
            s0, L = SEQ_BOUNDS[si]
            kT_sb, v_nat = kT_seq[si], v_seq[si]
            qh = p2.tile([P, 2048], bf, tag="qh")
            nc.sync.dma_start(qh[:, :L], qT_seq[si][h * P:(h + 1) * P, :])
            obuf = p2.tile([P, 2048], bf, tag="obuf")
            for qb in range(L // P):
                Q0 = qb * P
                kend = Q0 + P
                mstart = max(0, kend - 512)
                chunks = []
                c0 = 0
                while c0 < mstart:
                    w = min(512, mstart - c0)
                    chunks.append((c0, w, False))
                    c0 += w
                chunks.append((mstart, kend - mstart, True))

                pbuf = p2.tile([P, 2048], bf, tag="pbuf")
                acc = p3.tile([P, 8], f32, tag="acc")
                for ci, (c0, w, masked) in enumerate(chunks):
                    sps = psA.tile([P, 512], f32, tag="big")
                    nc.tensor.matmul(
                        sps[:, :w], lhsT=qh[:, Q0:Q0 + P],
                        rhs=kT_sb[:, c0:c0 + w], start=True, stop=True)
                    if not masked:
                        nc.scalar.activation(
                            pbuf[:, c0:c0 + w], sps[:, :w], AF.Exp,
                            scale=SCALE, accum_out=acc[:, ci:ci + 1])
                    else:
                        nc.scalar.activation(
                            pbuf[:, c0:c0 + w], sps[:, :w], AF.Exp, scale=SCALE)
                        nc.gpsimd.affine_select(
                            out=pbuf[:, c0:c0 + w], in_=pbuf[:, c0:c0 + w],
                            compare_op=ALU.is_ge, fill=0.0,
                            base=Q0 - c0, channel_multiplier=1,
                            pattern=[[-1, w]])
                        nc.vector.tensor_reduce(
                            out=acc[:, ci:ci + 1], in_=pbuf[:, c0:c0 + w],
                            axis=AX.X, op=ALU.add)
                nch = len(chunks)
                rs = p3.tile([P, 1], f32, tag="rs")
                nc.vector.tensor_reduce(out=rs[:], in_=acc[:, :nch],
                                        axis=AX.X, op=ALU.add)
                rrec = p3.tile([P, 1], f32, tag="rrec")
                nc.vector.reciprocal(rrec[:], rs[:])
                nc.vector.tensor_scalar_mul(pbuf[:, :kend], pbuf[:, :kend], rrec[:])

                ops = psB.tile([P, P], f32, tag="small")
                nkt = kend // P
                for kt in range(nkt):
                    pst = psB.tile([P, P], bf, tag="small")
                    nc.tensor.transpose(pst[:], pbuf[:, kt * P:(kt + 1) * P],
                                        ident[:])
                    pts = p3.tile([P, P], bf, tag="pts")
                    nc.any.tensor_copy(out=pts[:], in_=pst[:])
                    nc.tensor.matmul(
                        ops[:], lhsT=v_nat[:, kt, :], rhs=pts[:],
                        start=(kt == 0), stop=(kt == nkt - 1))
                nc.any.tensor_copy(out=obuf[:, Q0:kend], in_=ops[:])
            nc.sync.dma_start(attnT_seq[si][h * P:(h + 1) * P, :], obuf[:, :L])

        # ---------------- Phases 1+2 interleaved per sequence ----------------
        for si, (s0, L) in enumerate(SEQ_BOUNDS):
            q_dst_r = qT_seq[si].rearrange("(m p) t -> p m t", p=P)
            for lt in range(L // TW):
                phase1_tile(s0 + lt * TW, TW, kT_seq[si], lt * TW, v_seq[si],
                            q_dst_r, lt * TW)
            for h in range(QH):
                phase2_seq(si, h)

        # decode projections
        phase1_tile(DOFF, DECODE, kT_dec, 0, vdt,
                    qT_dec.rearrange("(m p) t -> p m t", p=P), 0)

        # decode q assembly: qdec_sb[:, 4s+h] = qT_dec[h*128:(h+1)*128, s]
        for h in range(QH):
            nc.gpsimd.dma_start(
                out=qdec_sb.rearrange("p (s h) -> p s h", h=QH)[:, :, h],
                in_=qT_dec[h * P:(h + 1) * P, :])

        # ---------------- Phase 3: decode attention ----------------
        dcos_sb = p1.tile([P, PAST], bf, tag="dcos")
        dsin_sb = p1.tile([P, PAST], bf, tag="dsin")
        nc.sync.dma_start(dcos_sb[:], dcos[:])
        nc.sync.dma_start(dsin_sb[:], dsin[:])

        for s in range(DECODE):
            kd = p2.tile([P, PAST], bf, tag="kd")
            nc.sync.dma_start(kd[:], kTc[s])
            rot = p2.tile([P, PAST], bf, tag="krot")
            nc.gpsimd.dma_start(out=rot[0:64, :], in_=kd[64:128, :])
            nc.gpsimd.dma_start(out=rot[64:128, :], in_=kd[0:64, :])
            nc.vector.tensor_tensor(kd[:], kd[:], dcos_sb[:], ALU.mult)
            nc.vector.tensor_tensor(rot[:], rot[:], dsin_sb[:], ALU.mult)
            nc.vector.tensor_tensor(kd[:], kd[:], rot[:], ALU.add)

            # v cache + ones column + (new v row | 1) as k-tile 16
            vd = p2.tile([P, NKT_D + 1, HD + 1], bf, tag="vd")
            nc.sync.dma_start(
                vd[:, :NKT_D, :HD], vc[s].rearrange("(kt p) d -> p kt d", p=P))
            nc.vector.memset(vd[:, :NKT_D, HD:HD + 1], 1.0)
            nc.gpsimd.dma_start(out=vd[0:1, NKT_D, :HD], in_=vdt[s:s + 1, :])
            nc.vector.memset(vd[0:1, NKT_D, HD:HD + 1], 1.0)

            stp = psB.tile([P, 68], f32, tag="small")
            for kt in range(NKT_D):
                nc.tensor.matmul(
                    stp[:, kt * QH:(kt + 1) * QH],
                    lhsT=kd[:, kt * P:(kt + 1) * P],
                    rhs=qdec_sb[:, s * QH:(s + 1) * QH], start=True, stop=True)
            nc.tensor.matmul(
                stp[0:1, 64:68], lhsT=kT_dec[:, s:s + 1],
                rhs=qdec_sb[:, s * QH:(s + 1) * QH], start=True, stop=True)
            pt = p3.tile([P, 68], bf, tag="ptd")
            nc.scalar.activation(pt[:, :64], stp[:, :64], AF.Exp, scale=SCALE)
            nc.scalar.activation(pt[0:1, 64:68], stp[0:1, 64:68], AF.Exp,
                                 scale=SCALE)

            ov = psB.tile([QH, HD + 1], f32, tag="small")
            for kt in range(NKT_D):
                nc.tensor.matmul(
                    ov[:], lhsT=pt[:, kt * QH:(kt + 1) * QH], rhs=vd[:, kt, :],
                    start=(kt == 0), stop=False)
            nc.tensor.matmul(ov[:], lhsT=pt[0:1, 64:68], rhs=vd[0:1, NKT_D, :],
                             start=False, stop=True)
            r4 = p3.tile([QH, 1], f32, tag="r4")
            nc.vector.reciprocal(r4[:], ov[:, HD:HD + 1])
            o4 = p3.tile([QH, HD], bf, tag="o4")
            nc.vector.tensor_scalar_mul(o4[:], ov[:, :HD], r4[:])
            nc.gpsimd.dma_start(out=odec_sb[s * QH:(s + 1) * QH, :], in_=o4[:])

        # transpose decode outputs into attnT_dec
        pst = psB.tile([P, P], bf, tag="small")
        nc.tensor.transpose(pst[:], odec_sb[:], ident[:])
        ot = p3.tile([P, P], bf, tag="otd")
        nc.any.tensor_copy(out=ot[:], in_=pst[:])
        otr = ot.rearrange("d (s h) -> d s h", h=QH)
        for h in range(QH):
            nc.gpsimd.dma_start(
                out=attnT_dec[h * P:(h + 1) * P, :], in_=otr[:, :, h])

        # ---------------- Phase 4: o_proj partial ----------------
        wo_sb = p1.tile([P, QH, HIDDEN], bf, tag="wbig")
        nc.sync.dma_start(wo_sb[:], woT_r[:])
        MH = HIDDEN // P  # 32 output blocks

        def phase4_tile(src_r, c0, W, t0):
            at = p2.tile([P, QH, TW], bf, tag="at")
            nc.sync.dma_start(at[:, :, :W], src_r[:, :, c0:c0 + W])
            for g in range(2):          # two DMA groups of 16 m-blocks
                omb = p2.tile([P, MH // 2, TW], bf, tag="omb")
                for mi in range(MH // 2):
                    m = g * (MH // 2) + mi
                    ps = psA.tile([P, 512], f32, tag="big")
                    for ks in range(QH):
                        nc.tensor.matmul(
                            ps[:, :W], lhsT=wo_sb[:, ks, m * P:(m + 1) * P],
                            rhs=at[:, ks, :W], start=(ks == 0),
                            stop=(ks == QH - 1))
                    nc.any.tensor_copy(out=omb[:, mi, :W], in_=ps[:, :W])
                nc.sync.dma_start(
                    outT_r[:, g * (MH // 2):(g + 1) * (MH // 2), t0:t0 + W],
                    omb[:, :, :W])

        for si, (s0, L) in enumerate(SEQ_BOUNDS):
            src_r = attnT_seq[si].rearrange("(o p) t -> p o t", p=P)
            for lt in range(L // TW):
                phase4_tile(src_r, lt * TW, TW, s0 + lt * TW)
        phase4_tile(attnT_dec.rearrange("(o p) t -> p o t", p=P), 0, DECODE, DOFF)

    nc.compile()
    return nc


_NC = None


def _get_program():
    global _NC
    if _NC is None:
        _NC = build_program()
    return _NC


def _rope_tables():
    inv_freq = 1.0 / (10000.0 ** (np.arange(0, HD, 2, dtype=np.float32) / HD))  # [64]
    pos_q = np.concatenate(
        [np.arange(L, dtype=np.float32) for L in PREFILLS]
        + [np.full(DECODE, float(PAST), np.float32)])                            # [T]
    ang_q = np.outer(inv_freq, pos_q)                                            # [64, T]
    qcos = np.concatenate([np.cos(ang_q), np.cos(ang_q)], axis=0)
    qsin = np.concatenate([-np.sin(ang_q), np.sin(ang_q)], axis=0)
    pos_d = np.arange(PAST, dtype=np.float32)
    ang_d = np.outer(inv_freq, pos_d)
    dcos = np.concatenate([np.cos(ang_d), np.cos(ang_d)], axis=0)
    dsin = np.concatenate([-np.sin(ang_d), np.sin(ang_d)], axis=0)
    return (qcos.astype(BF16), qsin.astype(BF16),
            dcos.astype(BF16), dsin.astype(BF16))


def make_in_maps(hidden_states, wq, wk, wv, wo, kv_cache_k, kv_cache_v):
    hidden_states = np.asarray(hidden_states)
    wq, wk, wv, wo = (np.asarray(a) for a in (wq, wk, wv, wo))
    kv_cache_k, kv_cache_v = np.asarray(kv_cache_k), np.asarray(kv_cache_v)

    hT = np.ascontiguousarray(hidden_states.astype(BF16).T)      # [4096, T]
    qcos, qsin, dcos, dsin = _rope_tables()
    in_maps = []
    for c in range(NCORES):
        wqT = np.ascontiguousarray(wq[c * ADIM:(c + 1) * ADIM, :].astype(BF16).T)
        wkT = np.ascontiguousarray(wk[c * HD:(c + 1) * HD, :].astype(BF16).T)
        wvT = np.ascontiguousarray(wv[c * HD:(c + 1) * HD, :].astype(BF16).T)
        woT = np.ascontiguousarray(wo[:, c * ADIM:(c + 1) * ADIM].astype(BF16).T)
        kTc = np.ascontiguousarray(
            kv_cache_k[:, :, c, :].astype(BF16).transpose(0, 2, 1))  # [32,128,2048]
        vcc = np.ascontiguousarray(kv_cache_v[:, :, c, :].astype(BF16))
        in_maps.append({
            "hT": hT, "wqT": wqT, "wkT": wkT, "wvT": wvT, "woT": woT,
            "kTc": kTc, "vc": vcc,
            "qcos": qcos, "qsin": qsin, "dcos": dcos, "dsin": dsin,
        })
    return in_maps


def combine_outputs(results):
    acc = np.zeros((HIDDEN, T), np.float32)
    for c in range(NCORES):
        acc += results[c]["outT"].astype(np.float32)
    return np.ascontiguousarray(acc.T)


def kernel(hidden_states, wq, wk, wv, wo, kv_cache_k, kv_cache_v):
    from concourse.bass_utils import run_bass_kernel_spmd

    nc = _get_program()
    in_maps = make_in_maps(hidden_states, wq, wk, wv, wo, kv_cache_k, kv_cache_v)
    res = run_bass_kernel_spmd(nc, in_maps, core_ids=list(range(NCORES)))
    return combine_outputs(res.results)



# revision 10
# speedup vs baseline: 1.2661x; 1.2661x over previous
"""Trainium2 Bass kernel for nn_LlamaAttention_61495341744411.

Sharding: tensor-parallel over heads across 8 NeuronCores.
  core c: q heads [4c, 4c+4), kv head c, wo cols [512c, 512c+512).
  Each core computes a full-token partial of out^T; host sums partials.

v2 design (per core, single SPMD program):
  - q/k/v projections in fp8(e4m3) DoubleRow with hi/lo error correction:
    X = Xh + Xl/32, W' = 32W = Wh + Wl;  W'X ~= Wh.Xh + Wl.Xh + (Wh/32).Xl
    computed as 3 DoubleRow matmuls per 2 k-tiles (1.33x bf16 FLOP rate,
    near-bf16 accuracy; validated vs reference in numpy).  The /32
    prescale is folded into the RoPE cos/sin tables (q,k) and the v copy.
  - attention computed in score-transposed orientation (S^T = K^T.q panels
    of 512), eliminating all P-transposes; PV accumulates oT directly;
    softmax denominators via ones-vector matmul; normalization by a
    rank-1 PE broadcast of 1/denom + one DVE multiply per (panel, head).
  - kv-cache K is RoPE'd on the host; decode attention is interleaved
    across the prefill panels so its DMA fully overlaps compute.
  - o_proj per panel from SBUF-resident attn outputs (no DRAM spills).
"""
import sys

if "/opt/trn_rl_repo" not in sys.path:
    sys.path.insert(0, "/opt/trn_rl_repo")

import numpy as np
import ml_dtypes

BF16 = ml_dtypes.bfloat16
E4M3 = ml_dtypes.float8_e4m3

PREFILLS = [1024, 1536, 2048, 512]
DOFF = sum(PREFILLS)            # 5120
DECODE = 32
PAST = 2048
HIDDEN = 4096
NQ, NKV, HD = 32, 8, 128
G = NQ // NKV                   # 4
T = DOFF + DECODE               # 5152
SCALE = 1.0 / float(np.sqrt(HD))
NCORES = 8
QH = NQ // NCORES               # 4 q heads per core
ADIM = QH * HD                  # 512
KS = HIDDEN // 128              # 32 contraction k-tiles
P = 128
TW = 256                        # projection token-tile width
PW = 512                        # attention q-panel width
NT = (T + TW - 1) // TW         # 21 token tiles (last = decode, 32 valid)
NKT_D = PAST // P               # 16 decode cache k-tiles
WS = 32.0                       # weight prescale

SEQ_BOUNDS = []
_off = 0
for _L in PREFILLS:
    SEQ_BOUNDS.append((_off, _L))
    _off += _L

# (si, panel, global t0) for every 512-token prefill panel
PANELS = []
for _si, (_s0, _L) in enumerate(SEQ_BOUNDS):
    for _p in range(_L // PW):
        PANELS.append((_si, _p, _s0 + _p * PW))


def build_program():
    import concourse.mybir as mybir
    import concourse.tile as tile
    from concourse import bacc
    from concourse.masks import make_identity
    from contextlib import ExitStack

    dt = mybir.dt
    AF = mybir.ActivationFunctionType
    ALU = mybir.AluOpType
    DR = mybir.MatmulPerfMode.DoubleRow
    f32 = dt.float32
    bf = dt.bfloat16
    f8 = dt.float8e4

    nc = bacc.Bacc(None, target_bir_lowering=False, debug=False)

    ht8 = nc.dram_tensor("ht8", [NT, P, KS, 2, TW], f8, kind="ExternalInput")
    whq = nc.dram_tensor("whq", [P, KS, ADIM], f8, kind="ExternalInput")
    wcq = nc.dram_tensor("wcq", [P, KS, 2, ADIM], f8, kind="ExternalInput")
    whk = nc.dram_tensor("whk", [P, KS, HD], f8, kind="ExternalInput")
    wck = nc.dram_tensor("wck", [P, KS, 2, HD], f8, kind="ExternalInput")
    whv = nc.dram_tensor("whv", [P, KS, HD], f8, kind="ExternalInput")
    wcv = nc.dram_tensor("wcv", [P, KS, 2, HD], f8, kind="ExternalInput")
    wod = nc.dram_tensor("wod", [P, QH, HIDDEN], bf, kind="ExternalInput")
    kTc = nc.dram_tensor("kTc", [DECODE, HD, PAST], bf, kind="ExternalInput")
    vcn = nc.dram_tensor("vcn", [DECODE, P, NKT_D, HD + 1], bf,
                         kind="ExternalInput")
    qcos = nc.dram_tensor("qcos", [HD, T], bf, kind="ExternalInput")
    qsin = nc.dram_tensor("qsin", [HD, T], bf, kind="ExternalInput")
    outT = nc.dram_tensor("outT", [HIDDEN, T], bf, kind="ExternalOutput")
    outT_r = outT.rearrange("(o p) t -> p o t", p=P)    # [128, 32, T]

    with ExitStack() as ctx:
        tc = ctx.enter_context(tile.TileContext(nc))
        p1 = ctx.enter_context(tc.tile_pool(name="p1", bufs=1))
        pseq = ctx.enter_context(tc.tile_pool(name="pseq", bufs=2))
        ppan = ctx.enter_context(tc.tile_pool(name="ppan", bufs=2))
        pht = ctx.enter_context(tc.tile_pool(name="pht", bufs=2))
        ppt = ctx.enter_context(tc.tile_pool(name="ppt", bufs=3))
        pdec = ctx.enter_context(tc.tile_pool(name="pdec", bufs=1))
        pvd = ctx.enter_context(tc.tile_pool(name="pvd", bufs=1))
        pd1 = ctx.enter_context(tc.tile_pool(name="pd1", bufs=1))
        pmb = ctx.enter_context(tc.tile_pool(name="pmb", bufs=1))
        psm = ctx.enter_context(tc.tile_pool(name="psm", bufs=2))
        psS = ctx.enter_context(tc.tile_pool(name="psS", bufs=2, space="PSUM"))
        psO = ctx.enter_context(tc.tile_pool(name="psO", bufs=2, space="PSUM"))
        psD = ctx.enter_context(tc.tile_pool(name="psD", bufs=1, space="PSUM"))
        psA = ctx.enter_context(tc.tile_pool(name="psA", bufs=3, space="PSUM"))

        ident = p1.tile([P, P], bf, tag="ident")
        make_identity(nc, ident)
        ones_c = p1.tile([P, 1], bf, tag="ones_c")
        nc.vector.memset(ones_c[:], 1.0)
        ones_r = p1.tile([1, P], bf, tag="ones_r")
        nc.vector.memset(ones_r[:], 1.0)

        # ---- weights (k/v first so the first matmuls start early) ----
        wk_h = p1.tile([P, KS, HD], f8, tag="wk_h")
        nc.sync.dma_start(wk_h[:], whk[:])
        wk_c = p1.tile([P, KS, 2, HD], f8, tag="wk_c")
        nc.sync.dma_start(wk_c[:], wck[:])
        wv_h = p1.tile([P, KS, HD], f8, tag="wv_h")
        nc.sync.dma_start(wv_h[:], whv[:])
        wv_c = p1.tile([P, KS, 2, HD], f8, tag="wv_c")
        nc.sync.dma_start(wv_c[:], wcv[:])
        wq_h = p1.tile([P, KS, ADIM], f8, tag="wq_h")
        nc.sync.dma_start(wq_h[:], whq[:])
        wq_c = p1.tile([P, KS, 2, ADIM], f8, tag="wq_c")
        nc.sync.dma_start(wq_c[:], wcq[:])
        wo_sb = p1.tile([P, QH, HIDDEN], bf, tag="wo")
        nc.sync.dma_start(wo_sb[:], wod[:])

        # per-seq resident k / v (rotating, sized for the longest seq)
        kT_seq = {}
        v_seq = {}

        # decode persistent tiles
        kT_dec = p1.tile([P, DECODE], bf, tag="kTdec")
        qdec_t = p1.tile([P, QH, DECODE], bf, tag="qdect")
        qdec_sb = p1.tile([P, P], bf, tag="qdec")
        vdt = p1.tile([DECODE, HD], bf, tag="vdt")
        odec_sb = p1.tile([P, HD], bf, tag="odec")
        aT_dec = p1.tile([P, QH, DECODE], bf, tag="aTdec")

        def proj_block(ps, wh, wc, ht, W):
            """fp8 DoubleRow projection of one 128-wide output block."""
            for j in range(KS // 2):
                nc.tensor.matmul(
                    ps[:, :W], lhsT=wh[:, 2 * j:2 * j + 2, :],
                    rhs=ht[:, 2 * j:2 * j + 2, 0, :W],
                    start=(j == 0), stop=False, perf_mode=DR)
            for kt in range(KS):
                nc.tensor.matmul(
                    ps[:, :W], lhsT=wc[:, kt, :, :],
                    rhs=ht[:, kt, :, :W],
                    start=False, stop=(kt == KS - 1), perf_mode=DR)

        def ph1_tile(ti, W, kT_dst, kcol0, v_dst, q_dst, qcol0):
            """Projections + rope for token tile ti (W valid cols).
            kT_dst[:, kcol0:+W] gets roped k;  q_dst [P, QH, *] gets roped
            q at qcol0;  v_dst: prefill -> v_nat [P, kt, HD+1] at k-tile
            kcol0//P (W=256), decode -> vdt [DECODE, HD] (W=32)."""
            t0 = ti * TW
            ht = pht.tile([P, KS, 2, TW], f8, tag="ht")
            nc.sync.dma_start(ht[:], ht8[ti])
            ct = pht.tile([P, TW], bf, tag="cos")
            st = pht.tile([P, TW], bf, tag="sin")
            nc.sync.dma_start(ct[:, :W], qcos[:, t0:t0 + W])
            nc.sync.dma_start(st[:, :W], qsin[:, t0:t0 + W])

            NB = QH + 1
            xq = pht.tile([P, NB, TW], bf, tag="xq")
            # k first (weights arrive first), then v, then q heads
            ps = psA.tile([P, 512], f32, tag="psA")
            proj_block(ps, wk_h, wk_c, ht, W)
            nc.scalar.activation(xq[:, QH, :W], ps[:, :W], AF.Copy)

            ps = psA.tile([P, 512], f32, tag="psA")
            proj_block(ps, wv_h, wv_c, ht, W)
            vt = pht.tile([P, TW], bf, tag="vt")
            nc.scalar.activation(vt[:, :W], ps[:, :W], AF.Copy,
                                 scale=1.0 / WS)
            if W == TW:
                for j in range(TW // P):
                    pst = psA.tile([P, P], bf, tag="psA")
                    nc.tensor.transpose(pst[:], vt[:, j * P:(j + 1) * P],
                                        ident[:])
                    nc.vector.tensor_copy(
                        out=v_dst[:, kcol0 // P + j, :HD], in_=pst[:])
            else:  # decode tile: W == 32
                pst = psA.tile([P, P], bf, tag="psA")
                nc.tensor.transpose(pst[:W, :], vt[:, :W], ident[:])
                nc.vector.tensor_copy(out=v_dst[:], in_=pst[:W, :])

            for m in range(QH):
                ps = psA.tile([P, 512], f32, tag="psA")
                proj_block(ps, wq_h[:, :, m * P:(m + 1) * P],
                           wq_c[:, :, :, m * P:(m + 1) * P], ht, W)
                nc.scalar.activation(xq[:, m, :W], ps[:, :W], AF.Copy)

            rotq = pht.tile([P, NB, TW], bf, tag="rotq")
            nc.gpsimd.dma_start(out=rotq[0:64, :, :W], in_=xq[64:128, :, :W])
            nc.gpsimd.dma_start(out=rotq[64:128, :, :W], in_=xq[0:64, :, :W])
            ct_b = ct[:, None, :W].to_broadcast((P, NB, W))
            st_b = st[:, None, :W].to_broadcast((P, NB, W))
            nc.vector.tensor_tensor(xq[:, :, :W], xq[:, :, :W], ct_b, ALU.mult)
            nc.vector.tensor_tensor(rotq[:, :, :W], rotq[:, :, :W], st_b,
                                    ALU.mult)
            nc.vector.tensor_tensor(q_dst[:, :, qcol0:qcol0 + W],
                                    xq[:, :QH, :W], rotq[:, :QH, :W], ALU.add)
            nc.vector.tensor_tensor(kT_dst[:, kcol0:kcol0 + W],
                                    xq[:, QH, :W], rotq[:, QH, :W], ALU.add)

        def ph2_panel(si, p, h, qT, aT):
            """Attention for (seq si, panel p, head h): S^T orientation."""
            kT_sb = kT_seq[si]
            v_nat = v_seq[si]
            nck = 4 * p + 4
            oT_ps = psO.tile([P, PW], f32, tag="psO")
            den = psD.tile([1, PW], f32, tag="psD")

            stage = []   # chunks with pending den+PV (software pipeline)

            def drain_one():
                ct_, off_, w_, pt_ = stage.pop(0)
                nc.tensor.matmul(den[0:1, off_:off_ + w_],
                                 lhsT=ones_c[:], rhs=pt_[:, off_:off_ + w_],
                                 start=(ct_ == 0), stop=(ct_ == nck - 1))
                nc.tensor.matmul(oT_ps[:, off_:off_ + w_],
                                 lhsT=v_nat[:, ct_, :HD],
                                 rhs=pt_[:, off_:off_ + w_],
                                 start=(ct_ == 0), stop=(ct_ == nck - 1))

            for ct in range(nck):
                off = max(0, (ct - 4 * p)) * P
                w = PW - off
                sps = psS.tile([P, PW], f32, tag="psS")
                nc.tensor.matmul(sps[:, off:off + w],
                                 lhsT=kT_sb[:, ct * P:(ct + 1) * P],
                                 rhs=qT[:, h, off:PW],
                                 start=True, stop=True)
                pt = ppt.tile([P, PW], bf, tag="pt")
                nc.scalar.activation(pt[:, off:off + w], sps[:, off:off + w],
                                     AF.Exp, scale=SCALE)
                if ct >= 4 * p:   # diagonal block: mask c > q within 128 cols
                    nc.gpsimd.affine_select(
                        out=pt[:, off:off + P], in_=pt[:, off:off + P],
                        compare_op=ALU.is_ge, fill=0.0,
                        base=0, channel_multiplier=-1, pattern=[[1, P]])
                stage.append((ct, off, w, pt))
                if len(stage) >= 2:
                    drain_one()
            while stage:
                drain_one()

            # normalize: rr = 1/den, partition-broadcast via SWDGE, then
            # aT[:, h, :] = oT * rr_b  (single DVE op)
            rr = psm.tile([1, PW], bf, tag="rr")
            with nc.allow_low_precision(reason="softmax denom bcast in bf16"):
                nc.vector.reciprocal(rr[:], den[0:1, :])
            rr_ps = psA.tile([P, PW], f32, tag="psA")
            nc.tensor.matmul(rr_ps[:], lhsT=ones_r[:], rhs=rr[:],
                             start=True, stop=True)
            rrb = psm.tile([P, PW], bf, tag="rrb")
            nc.scalar.activation(rrb[:], rr_ps[:], AF.Copy)
            nc.vector.tensor_tensor(aT[:, h, :], oT_ps[:], rrb[:], ALU.mult)

        def ph4_panel(aT, t0, W):
            """o_proj partial for W tokens starting at global t0."""
            for g in range(8):
                omb = pmb.tile([P, 4, PW], bf, tag="omb")
                for mi in range(4):
                    m = g * 4 + mi
                    ps = psA.tile([P, 512], f32, tag="psA")
                    for ks in range(QH):
                        nc.tensor.matmul(
                            ps[:, :W], lhsT=wo_sb[:, ks, m * P:(m + 1) * P],
                            rhs=aT[:, ks, :W], start=(ks == 0),
                            stop=(ks == QH - 1))
                    if mi % 2 == 0:
                        nc.scalar.activation(omb[:, mi, :W], ps[:, :W],
                                             AF.Copy)
                    else:
                        nc.vector.tensor_copy(out=omb[:, mi, :W],
                                              in_=ps[:, :W])
                nc.sync.dma_start(
                    outT_r[:, g * 4:(g + 1) * 4, t0:t0 + W], omb[:, :, :W])

        # ---------------- decode helpers ----------------
        dec_tiles = {}

        def decode_dma(s):
            kd = pdec.tile([P, PAST], bf, tag="kd")
            nc.sync.dma_start(kd[:], kTc[s])
            vd = pvd.tile([P, NKT_D + 1, HD + 1], bf, tag="vd")
            nc.sync.dma_start(vd[:, :NKT_D, :], vcn[s])
            dec_tiles[s] = (kd, vd)

        def decode_compute(s):
            kd, vd = dec_tiles.pop(s)
            nc.gpsimd.dma_start(out=vd[0:1, NKT_D, :HD], in_=vdt[s:s + 1, :])
            nc.vector.memset(vd[0:1, NKT_D, HD:HD + 1], 1.0)

            stp = psS.tile([P, PW], f32, tag="psS")
            for kt in range(NKT_D):
                nc.tensor.matmul(
                    stp[:, kt * QH:(kt + 1) * QH],
                    lhsT=kd[:, kt * P:(kt + 1) * P],
                    rhs=qdec_sb[:, s * QH:(s + 1) * QH], start=True, stop=True)
            nc.tensor.matmul(
                stp[0:1, 64:68], lhsT=kT_dec[:, s:s + 1],
                rhs=qdec_sb[:, s * QH:(s + 1) * QH], start=True, stop=True)
            pt = pd1.tile([P, PW], bf, tag="ptd")
            nc.scalar.activation(pt[:, :64], stp[:, :64], AF.Exp, scale=SCALE)
            nc.scalar.activation(pt[0:1, 64:68], stp[0:1, 64:68], AF.Exp,
                                 scale=SCALE)

            ov = psA.tile([QH, HD + 1], f32, tag="psA")
            for kt in range(NKT_D):
                nc.tensor.matmul(
                    ov[:], lhsT=pt[:, kt * QH:(kt + 1) * QH],
                    rhs=vd[:, kt, :], start=(kt == 0), stop=False)
            nc.tensor.matmul(ov[:], lhsT=pt[0:1, 64:68],
                             rhs=vd[0:1, NKT_D, :], start=False, stop=True)
            r4 = pd1.tile([QH, 1], f32, tag="r4")
            nc.vector.reciprocal(r4[:], ov[:, HD:HD + 1])
            o4 = pd1.tile([QH, HD], bf, tag="o4")
            nc.vector.tensor_scalar_mul(o4[:], ov[:, :HD], r4[:])
            nc.gpsimd.dma_start(out=odec_sb[s * QH:(s + 1) * QH, :], in_=o4[:])

        # ================= emission =================
        def seq_tiles(si, p):
            s0, L = SEQ_BOUNDS[si]
            kT_sb = kT_seq[si]
            v_nat = v_seq[si]
            qT = qT_pan[(si, p)]
            for half in range(2):
                t0 = s0 + p * PW + half * TW
                ph1_tile(t0 // TW, TW, kT_sb, p * PW + half * TW,
                         v_nat, qT, half * TW)

        qT_pan = {}

        def alloc_seq(si):
            kT_seq[si] = pseq.tile([P, 2048], bf, tag="kT",
                                   name=f"kT{si}")
            v_seq[si] = pseq.tile([P, 2048 // P, HD + 1], bf, tag="vn",
                                  name=f"vn{si}")

        def alloc_panel(si, p):
            qT_pan[(si, p)] = ppan.tile([P, QH, PW], bf, tag="qT",
                                        name=f"qT{si}_{p}")

        # prologue: seq0 panel0 projections, then decode projections
        alloc_seq(0)
        alloc_panel(0, 0)
        seq_tiles(0, 0)
        ph1_tile(NT - 1, DECODE, kT_dec, 0, vdt, qdec_t, 0)
        qd_r = qdec_sb.rearrange("p (s h) -> p s h", h=QH)
        for h in range(QH):
            nc.gpsimd.dma_start(out=qd_r[:, :, h], in_=qdec_t[:, h, :])

        # decode interleave schedule: 3 slots per panel
        dec_next_dma = 0
        dec_next_cmp = 0

        def decode_slot():
            nonlocal dec_next_dma, dec_next_cmp
            if dec_next_cmp < dec_next_dma:
                decode_compute(dec_next_cmp)
                dec_next_cmp += 1
            if dec_next_dma < DECODE:
                decode_dma(dec_next_dma)
                dec_next_dma += 1

        for step, (si, p, t0g) in enumerate(PANELS):
            # ph1 one panel ahead
            if step + 1 < len(PANELS):
                nsi, np_, _ = PANELS[step + 1]
                if np_ == 0:
                    alloc_seq(nsi)
                alloc_panel(nsi, np_)
                seq_tiles(nsi, np_)
            aT = ppan.tile([P, QH, PW], bf, tag="aT", name=f"aT{si}_{p}")
            qT = qT_pan.pop((si, p))
            for h in range(QH):
                ph2_panel(si, p, h, qT, aT)
                if h in (1, 3):
                    decode_slot()
            ph4_panel(aT, t0g, PW)
            decode_slot()

        # remaining decode
        while dec_next_cmp < DECODE:
            if dec_next_dma < DECODE:
                decode_dma(dec_next_dma)
                dec_next_dma += 1
            decode_compute(dec_next_cmp)
            dec_next_cmp += 1

        # decode outputs -> aT_dec -> o_proj
        pst = psA.tile([P, P], bf, tag="psA")
        nc.tensor.transpose(pst[:], odec_sb[:], ident[:])
        ot = pd1.tile([P, P], bf, tag="otd")
        nc.vector.tensor_copy(out=ot[:], in_=pst[:])
        otr = ot.rearrange("d (s h) -> d s h", h=QH)
        for h in range(QH):
            nc.gpsimd.dma_start(out=aT_dec[:, h, :], in_=otr[:, :, h])
        ph4_panel(aT_dec, DOFF, DECODE)

    nc.compile()
    return nc


_NC = None


def _get_program():
    global _NC
    if _NC is None:
        _NC = build_program()
    return _NC


def _rope_tables():
    """cos/sin tables [128, T] with the 1/WS projection descale folded in."""
    inv_freq = 1.0 / (10000.0 ** (np.arange(0, HD, 2, dtype=np.float32) / HD))
    pos_q = np.concatenate(
        [np.arange(L, dtype=np.float32) for L in PREFILLS]
        + [np.full(DECODE, float(PAST), np.float32)])                 # [T]
    ang = np.outer(inv_freq, pos_q)                                   # [64, T]
    qcos = np.concatenate([np.cos(ang), np.cos(ang)], axis=0) / WS
    qsin = np.concatenate([-np.sin(ang), np.sin(ang)], axis=0) / WS
    return qcos.astype(BF16), qsin.astype(BF16)


def _split_w(wT):
    """wT [4096, M] f32 -> (wh [128, 32, M], wc [128, 32, 2, M]) fp8."""
    M = wT.shape[1]
    wp = (wT * WS).astype(np.float32)
    wh = wp.astype(E4M3)
    wl = (wp - wh.astype(np.float32)).astype(E4M3)
    wd = (wh.astype(np.float32) / WS).astype(E4M3)
    wh3 = np.ascontiguousarray(
        wh.reshape(KS, P, M).transpose(1, 0, 2))
    wc = np.ascontiguousarray(
        np.stack([wl.reshape(KS, P, M), wd.reshape(KS, P, M)],
                 axis=2).transpose(1, 0, 2, 3))
    return wh3, wc


def make_in_maps(hidden_states, wq, wk, wv, wo, kv_cache_k, kv_cache_v):
    hidden_states = np.asarray(hidden_states, np.float32)
    wq, wk, wv, wo = (np.asarray(a, np.float32) for a in (wq, wk, wv, wo))
    kv_cache_k = np.asarray(kv_cache_k, np.float32)
    kv_cache_v = np.asarray(kv_cache_v, np.float32)

    # hidden^T split into fp8 hi/lo, packed tile-major [NT,128,32,2,256]
    hT = hidden_states.T                                   # [4096, T]
    pad = NT * TW - T
    hTp = np.pad(hT, ((0, 0), (0, pad)))
    xh = hTp.astype(E4M3)
    xl = ((hTp - xh.astype(np.float32)) * WS).astype(E4M3)
    # [4096, NTT] -> [32, 128, NT, 256] -> [NT, 128, 32, 256]
    def pack(a):
        return a.reshape(KS, P, NT, TW).transpose(2, 1, 0, 3)
    ht8 = np.ascontiguousarray(
        np.stack([pack(xh), pack(xl)], axis=3))            # [NT,128,32,2,256]

    qcos, qsin = _rope_tables()

    # host-side RoPE of the k cache (reference semantics, fp32)
    inv_freq = 1.0 / (10000.0 ** (np.arange(0, HD, 2, dtype=np.float32) / HD))
    kpos = np.arange(PAST, dtype=np.float32)
    ang = np.outer(kpos, inv_freq)                          # [PAST, 64]
    cos = np.concatenate([np.cos(ang), np.cos(ang)], axis=1)[None, :, None, :]
    sin = np.concatenate([np.sin(ang), np.sin(ang)], axis=1)[None, :, None, :]
    rot = np.concatenate([-kv_cache_k[..., HD // 2:],
                          kv_cache_k[..., :HD // 2]], axis=-1)
    kroped = kv_cache_k * cos + rot * sin                   # [D, PAST, 8, HD]

    in_maps = []
    for c in range(NCORES):
        wh_q, wc_q = _split_w(wq[c * ADIM:(c + 1) * ADIM, :].T)
        wh_k, wc_k = _split_w(wk[c * HD:(c + 1) * HD, :].T)
        wh_v, wc_v = _split_w(wv[c * HD:(c + 1) * HD, :].T)
        wod = np.ascontiguousarray(
            wo[:, c * ADIM:(c + 1) * ADIM].T.reshape(QH, P, HIDDEN)
            .transpose(1, 0, 2).astype(BF16))               # [128, 4, 4096]
        kTcc = np.ascontiguousarray(
            kroped[:, :, c, :].transpose(0, 2, 1).astype(BF16))  # [D,128,PAST]
        # v cache -> [D, 128, 16, 129] with ones column baked in
        vcc = kv_cache_v[:, :, c, :].reshape(DECODE, NKT_D, P, HD)
        vcc = vcc.transpose(0, 2, 1, 3)                     # [D, 128, 16, HD]
        vcn = np.concatenate(
            [vcc, np.ones((DECODE, P, NKT_D, 1), np.float32)], axis=3)
        vcn = np.ascontiguousarray(vcn.astype(BF16))
        in_maps.append({
            "ht8": ht8, "whq": wh_q, "wcq": wc_q, "whk": wh_k, "wck": wc_k,
            "whv": wh_v, "wcv": wc_v, "wod": wod, "kTc": kTcc, "vcn": vcn,
            "qcos": qcos, "qsin": qsin,
        })
    return in_maps


def combine_outputs(results):
    acc = np.zeros((HIDDEN, T), np.float32)
    for c in range(NCORES):
        acc += results[c]["outT"].astype(np.float32)
    return np.ascontiguousarray(acc.T)


def kernel(hidden_states, wq, wk, wv, wo, kv_cache_k, kv_cache_v):
    from concourse.bass_utils import run_bass_kernel_spmd

    nc = _get_program()
    in_maps = make_in_maps(hidden_states, wq, wk, wv, wo, kv_cache_k,
                           kv_cache_v)
    res = run_bass_kernel_spmd(nc, in_maps, core_ids=list(range(NCORES)))
    return combine_outputs(res.results)
